# revision 14
# baseline (speedup 1.0000x reference)
# Trainium2 Bass kernel for nn_CustomConv2D_57200374448719:
#   data [32,128,64,64] f32 (NCHW) conv weights [256,128,3,3] (OIHW),
#   VALID, stride 1 -> out [32,256,62,62] f32.
#
# Strategy: data-parallel over batch across 8 NeuronCores (4 images per
# core), weights replicated. Per core, implicit GEMM with C_in=128 on the
# SBUF partition axis: for each image / C_out half (128) / group of 8
# output rows, accumulate 9 matmuls (one per 3x3 tap, K=128, N=rows*62)
# into one PSUM bank. The shifted conv windows are strided access
# patterns on the resident image tiles (3D moving AP [128, rows, 62],
# row stride 64), so no im2col copy is materialized and no garbage
# columns are streamed. Matmuls run in bfloat16 by default: bf16 lowers
# to separate LDWEIGHTS+MATMUL with fast-weight-load, and the PE's
# reorder window pulls the next LDWEIGHTS into the background weight
# buffer during the current matmul's streaming - the stationary loads
# that cost ~20% extra on the fp32r self-loading path are fully hidden.
# Accumulation is fp32 in PSUM; measured rel err ~2e-3.
#
# Startup-latency hiding: weights are loaded as two per-co-half chunks
# and each image as halo'd row chunks, all on the sync-engine HWDGE
# ring, ordered so the first row-group's dependencies (first weight
# chunk + first image chunk) land as early as possible. While those
# DMAs are in flight, 8 dummy matmuls on a memset scratch tile keep the
# PE busy so the HAM clock gate releases (1.2 -> 2.4 GHz) right as real
# work begins. Output stores go on the scalar-engine HWDGE ring per
# row-group so they stream out during compute; the final row-group's
# store is split in half across the scalar and sync rings to shorten
# the drain tail.
import numpy as np

N_CORES = 8
B, CIN, H, W = 32, 128, 64, 64
COUT, KH, KW = 256, 3, 3
OH, OW = H - KH + 1, W - KW + 1  # 62, 62
BPC = B // N_CORES  # images per core
# Startup is input-bandwidth-bound: the first matmul releases only after
# its chunk+weights have fully drained (~280 GB/s aggregate) plus ~1.7us
# of completion-semaphore latency. Two small 4-row groups up front keep
# the first dependencies tiny (96 KB chunk + 96 KB tap weights), then
# 8-row groups once the pipe is primed.
ROW_GROUPS = [(0, 4), (4, 4)] + [(r0, 8) for r0 in range(8, 56, 8)] + [(56, 6)]
# image row chunks (with conv halo): [0,6) serves group (0,4), [4,10)
# serves (4,4), [8,18) serves (8,8), [16,34) serves (16,8)+(24,8),
# [32,64) serves the rest.
CHUNKS = [(0, 6), (4, 6), (8, 10), (16, 18), (32, 32)]
# The first real matmul can't release before ~10.5us: first DMA trigger
# (~6.5, after the framework preamble) + ~1.5us HWDGE arming + ~0.7us
# descriptor drain + ~1.7us completion-semaphore latency. Dummy matmuls
# bridge PE-busy start (~7.8us, after the scratch memset) to that point
# so the HAM clock gate is released (1.2 -> 2.4 GHz) when real work
# starts. 7 emitted = 6 executed x ~0.43us cold.
WARMUP_MM = 7
# the very last (image, co-half) splits its final 6-row group into 4+2
# so the kernel-end store drain is 2 rows (63KB) instead of 6 (190KB)
ROW_GROUPS_LAST = ROW_GROUPS[:-1] + [(56, 4), (60, 2)]

_cache = {}


def build_nc(mm_dtype_name="bfloat16"):
    import concourse.bacc as bacc
    import concourse.mybir as mybir
    import concourse.tile as tile

    mm_dt = getattr(mybir.dt, mm_dtype_name)
    f32 = mybir.dt.float32

    nc = bacc.Bacc("TRN2", target_bir_lowering=False, debug=False, num_devices=N_CORES)
    data_in = nc.dram_tensor("data", [BPC, CIN, H, W], mm_dt, kind="ExternalInput").ap()
    # wt[ci, g*(9*128) + t*128 + co'] = weights[g*128+co', ci, ky, kx], t=ky*3+kx
    w_in = nc.dram_tensor("wt", [CIN, KH * KW * COUT], mm_dt, kind="ExternalInput").ap()
    out = nc.dram_tensor("out", [BPC, COUT, OH, OW], f32, kind="ExternalOutput").ap()
    WG = KH * KW * 128  # columns per co-half weight chunk

    with tile.TileContext(nc) as tc:
        with (
            tc.tile_pool(name="wpool", bufs=1) as wpool,
            tc.tile_pool(name="scr", bufs=1) as spool,
            tc.tile_pool(name="dpool", bufs=2) as dpool,
            tc.tile_pool(name="opool", bufs=6) as opool,
            tc.tile_pool(name="psum", bufs=8, space="PSUM") as ppool,
        ):
            # PE warm-up: the HAM clock gate holds the PE at 1.2 GHz until
            # one ~3.4us activity window has been busy, and the first ~9.5us
            # here are DMA-bound (preamble + weight/image loads). Run dummy
            # matmuls on memset scratch spanning that window so real
            # matmuls start at 2.4 GHz. (fp32 dummies don't work: they
            # lower to LOW_HIGH pairs, take 2-3x longer than budgeted, and
            # the sustained draw downclocks the whole stream.)
            # memset on the vector engine: its sequencer clears the
            # framework preamble earliest, so the warm-up (and with it
            # the HAM busy clock) starts as soon as possible
            if mm_dtype_name == "float32r":
                wscr = spool.tile([128, 512], f32)
                nc.vector.memset(wscr[:], 0.0)
                wsr = wscr[:].bitcast(mm_dt)
            else:
                wscr = spool.tile([128, 512], mm_dt)
                nc.vector.memset(wscr[:], 0.0)
                wsr = wscr[:]
            # the warm-up PSUM tile shares the main pool's slots (it is
            # long released by the time the 8th real group needs its bank)
            wps = ppool.tile([128, 512], f32, tag="ps")
            for _ in range(WARMUP_MM):
                nc.tensor.matmul(wps[:], wsr[:, :128], wsr[:], start=True, stop=True)

            # co-half 0's weights stream as three 96 KB tap-triples on the
            # scalar ring (concurrent with the image chunks on the sync
            # ring) so tap t's weights land just ahead of its first
            # matmul; co-half 1 follows as one chunk (needed ~14us in).
            # Image-0 chunk 0 goes FIRST on the sync ring: the first
            # matmul's gating transfers are only 96+96 KB.
            wt_g0 = [
                wpool.tile([CIN, 3 * 128], mm_dt, tag=f"wt0_{i}", name=f"wt_g0_{i}")
                for i in range(3)
            ]
            wt_g1 = wpool.tile([CIN, WG], mm_dt, tag="wt1")

            def wslice(g, t):
                if g == 1:
                    return wt_g1[:, t * 128 : (t + 1) * 128]
                return wt_g0[t // 3][:, (t % 3) * 128 : (t % 3 + 1) * 128]

            ct00 = dpool.tile([CIN, CHUNKS[0][1] * W], mm_dt, tag="d0")
            nc.sync.dma_start(
                ct00[:],
                data_in[0].rearrange("c h w -> c (h w)")[:, : CHUNKS[0][1] * W],
            )
            for i in range(3):
                nc.scalar.dma_start(
                    wt_g0[i][:], w_in[:, i * 3 * 128 : (i + 1) * 3 * 128]
                )

            dtiles = []
            for n in range(BPC):
                chunks = []
                flat = data_in[n].rearrange("c h w -> c (h w)")
                for ci, (c0, crows) in enumerate(CHUNKS):
                    if n == 0 and ci == 0:
                        chunks.append(ct00)
                        continue
                    ct = dpool.tile([CIN, crows * W], mm_dt, tag=f"d{ci}")
                    nc.sync.dma_start(ct[:], flat[:, c0 * W : (c0 + crows) * W])
                    chunks.append(ct)
                    if n == 0 and ci == 2:
                        nc.scalar.dma_start(wt_g1[:], w_in[:, WG:])
                dtiles.append(chunks)

            def rhs_for(chunks, r0, rows, t):
                ci = next(
                    i
                    for i, (c0, crows) in enumerate(CHUNKS)
                    if r0 >= c0 and r0 + rows + KH - 1 <= c0 + crows
                )
                hr0 = r0 - CHUNKS[ci][0]
                ky, kx = divmod(t, KW)
                c3 = chunks[ci][:].rearrange("p (r w) -> p r w", w=W)
                return c3[:, hr0 + ky : hr0 + ky + rows, kx : kx + OW]

            def out_rows(n, g, a, b):
                return out[n].rearrange("c h w -> c (h w)")[
                    g * 128 : (g + 1) * 128, a * OW : b * OW
                ]

            def evacuate(n, g, r, r0, rows, ps, tail):
                ot = opool.tile([128, 8 * OW], f32, tag="ot")
                if not tail:
                    # vector picks up the PSUM-done semaphore noticeably
                    # faster than scalar; alternate so neither engine backs
                    # up, and the stores stream on the scalar ring
                    if r % 2 == 0:
                        nc.vector.tensor_copy(ot[:, : rows * OW], ps[:])
                    else:
                        nc.scalar.copy(ot[:, : rows * OW], ps[:])
                    nc.scalar.dma_start(out_rows(n, g, r0, r0 + rows), ot[:, : rows * OW])
                else:
                    # the last two row-groups: copies on vector (fast sem
                    # pickup), stores on separate rings so the final 2-row
                    # store's descriptor gen + drain is all that remains
                    # after the last matmul group
                    nc.vector.tensor_copy(ot[:, : rows * OW], ps[:])
                    ring = nc.sync if rows == 2 else nc.scalar
                    ring.dma_start(out_rows(n, g, r0, r0 + rows), ot[:, : rows * OW])

            for n in range(BPC):
                chunks = dtiles[n]
                for g in range(COUT // 128):
                    is_last_ng = n == BPC - 1 and g == COUT // 128 - 1
                    groups = ROW_GROUPS_LAST if is_last_ng else ROW_GROUPS
                    for r, (r0, rows) in enumerate(groups):
                        ps = ppool.tile([128, rows * OW], f32, tag="ps")
                        for t in range(KH * KW):
                            nc.tensor.matmul(
                                ps[:],
                                wslice(g, t),
                                rhs_for(chunks, r0, rows, t),
                                start=(t == 0),
                                stop=(t == KH * KW - 1),
                            )
                        evacuate(n, g, r, r0, rows, ps, is_last_ng and rows < 8)
    nc.compile()
    return nc


def _get_nc(mm_dtype_name="bfloat16"):
    if mm_dtype_name not in _cache:
        _cache[mm_dtype_name] = build_nc(mm_dtype_name)
    return _cache[mm_dtype_name]


def _get_runner(mm_dtype_name="bfloat16"):
    """Build the 8-core PJRT executable once and cache it: repeat kernel()
    calls then skip bass2jax's per-call jit re-trace (~6s each)."""
    key = ("runner", mm_dtype_name)
    if key in _cache:
        return _cache[key]

    import jax
    import jax.core
    from jax.experimental.shard_map import shard_map
    from jax.sharding import Mesh, PartitionSpec

    import concourse.mybir as mybir
    from concourse import bass2jax

    nc = _get_nc(mm_dtype_name)
    bass2jax.install_neuronx_cc_hook()

    partition_name = nc.partition_id_tensor.name if nc.partition_id_tensor else None
    in_names, out_names, out_avals = [], [], []
    for alloc in nc.m.functions[0].allocations:
        if not isinstance(alloc, mybir.MemoryLocationSet):
            continue
        name = alloc.memorylocations[0].name
        if alloc.kind == "ExternalInput":
            if name != partition_name:
                in_names.append(name)
        elif alloc.kind == "ExternalOutput":
            out_names.append(name)
            out_avals.append(
                jax.core.ShapedArray(
                    tuple(alloc.tensor_shape), mybir.dt.np(alloc.dtype)
                )
            )
    n_params, n_outs = len(in_names), len(out_names)
    all_names = in_names + out_names
    if partition_name is not None:
        all_names = all_names + [partition_name]
    donate = tuple(range(n_params, n_params + n_outs))

    def _body(*args):
        operands = list(args)
        if partition_name is not None:
            operands.append(bass2jax.partition_id_tensor())
        outs = bass2jax._bass_exec_p.bind(
            *operands,
            out_avals=tuple(out_avals),
            in_names=tuple(all_names),
            out_names=tuple(out_names),
            lowering_input_output_aliases=(),
            sim_require_finite=True,
            sim_require_nnan=True,
            nc=nc,
        )
        return tuple(outs)

    devices = jax.devices()[:N_CORES]
    mesh = Mesh(np.asarray(devices), ("core",))
    sharded = jax.jit(
        shard_map(
            _body,
            mesh=mesh,
            in_specs=(PartitionSpec("core"),) * (n_params + n_outs),
            out_specs=(PartitionSpec("core"),) * n_outs,
            check_rep=False,
        ),
        donate_argnums=donate,
        keep_unused=True,
    )
    runner = (in_names, out_names, out_avals, sharded)
    _cache[key] = runner
    return runner


def _np_in_dtype(mm_dtype_name):
    if mm_dtype_name == "bfloat16":
        import ml_dtypes

        return ml_dtypes.bfloat16
    if mm_dtype_name == "float16":
        return np.float16
    return np.float32


def _prep_weights(weights, np_dt):
    # [co, ci, ky, kx] -> [ci][t=ky*3+kx][g][co'] -> [ci][g][t][co'] flat
    w4 = np.asarray(weights, dtype=np.float32).transpose(1, 2, 3, 0)  # ci,ky,kx,co
    w4 = w4.reshape(CIN, KH * KW, COUT // 128, 128).transpose(0, 2, 1, 3)
    return np.ascontiguousarray(w4, dtype=np_dt).reshape(CIN, KH * KW * COUT)


def kernel(data: np.ndarray, weights: np.ndarray, _dtype="bfloat16") -> np.ndarray:
    np_dt = _np_in_dtype(_dtype)
    data = np.ascontiguousarray(np.asarray(data), dtype=np_dt)
    wt = _prep_weights(weights, np_dt)

    in_names, out_names, out_avals, sharded = _get_runner(_dtype)
    # shard_map splits axis 0 across the 8 cores: the global batch-sharded
    # arrays are exactly the full input (batch 32 -> 4 per core) and the
    # per-core-replicated weights tiled 8x on axis 0.
    globals_ = {
        "data": data.reshape(N_CORES * BPC, CIN, H, W),
        "wt": np.tile(wt, (N_CORES, 1)),
    }
    args = [globals_[n] for n in in_names] + [
        np.zeros((N_CORES * av.shape[0], *av.shape[1:]), av.dtype)
        for av in out_avals
    ]
    outs = sharded(*args)
    return np.asarray(outs[out_names.index("out")])


# revision 17
# speedup vs baseline: 1.0156x; 1.0156x over previous
# Trainium2 Bass kernel for nn_CustomConv2D_57200374448719:
#   data [32,128,64,64] f32 (NCHW) conv weights [256,128,3,3] (OIHW),
#   VALID, stride 1 -> out [32,256,62,62] f32.
#
# Strategy: data-parallel over batch across 8 NeuronCores (4 images per
# core), weights replicated. Per core, implicit GEMM with C_in=128 on the
# SBUF partition axis: for each image / C_out half (128) / group of 8
# output rows, accumulate 9 matmuls (one per 3x3 tap, K=128, N=rows*62)
# into one PSUM bank. The shifted conv windows are strided access
# patterns on the resident image tiles (3D moving AP [128, rows, 62],
# row stride 64), so no im2col copy is materialized and no garbage
# columns are streamed. Matmuls run in bfloat16 by default: bf16 lowers
# to separate LDWEIGHTS+MATMUL with fast-weight-load, and the PE's
# reorder window pulls the next LDWEIGHTS into the background weight
# buffer during the current matmul's streaming - the stationary loads
# that cost ~20% extra on the fp32r self-loading path are fully hidden.
# Accumulation is fp32 in PSUM; measured rel err ~2e-3.
#
# Startup-latency hiding: weights are loaded as two per-co-half chunks
# and each image as halo'd row chunks, all on the sync-engine HWDGE
# ring, ordered so the first row-group's dependencies (first weight
# chunk + first image chunk) land as early as possible. While those
# DMAs are in flight, 8 dummy matmuls on a memset scratch tile keep the
# PE busy so the HAM clock gate releases (1.2 -> 2.4 GHz) right as real
# work begins. Output stores go on the scalar-engine HWDGE ring per
# row-group so they stream out during compute; the final row-group's
# store is split in half across the scalar and sync rings to shorten
# the drain tail.
import numpy as np

N_CORES = 8
B, CIN, H, W = 32, 128, 64, 64
COUT, KH, KW = 256, 3, 3
OH, OW = H - KH + 1, W - KW + 1  # 62, 62
BPC = B // N_CORES  # images per core
ROW_GROUPS = [(r0, min(8, OH - r0)) for r0 in range(0, OH, 8)]  # 7x8 + 1x6
# image row chunks (with conv halo): rows [0,10) serves row-group 0,
# [8,18) serves 1, [16,34) serves 2-3, [32,64) serves 4-7. The first two
# are small so the first matmuls' DMA dependencies land early. Startup
# is input-bandwidth-bound (~300 GB/s aggregate + ~1.7us completion-
# semaphore latency per transfer), so finer staging is zero-sum: it
# releases the first matmul earlier but starves later groups instead.
CHUNKS = [(0, 10), (8, 10), (16, 18), (32, 32)]
# The first real matmul can't release before ~10.7us: first DMA trigger
# (~6.5, after the framework preamble) + ~1.5us HWDGE arming + ~1us
# descriptor drain + ~1.7us completion-semaphore latency. Dummy matmuls
# bridge PE-busy start (~7.7us, after the scratch memset) to that point
# so the HAM clock gate is released (1.2 -> 2.4 GHz) when real work
# starts. 8 emitted = 7 executed x ~0.43us cold.
WARMUP_MM = 8
# the very last (image, co-half) splits its final 6-row group into 4+2
# so the kernel-end store drain is 2 rows (63KB) instead of 6 (190KB)
ROW_GROUPS_LAST = ROW_GROUPS[:-1] + [(56, 4), (60, 2)]

_cache = {}


def build_nc(mm_dtype_name="bfloat16"):
    import concourse.bacc as bacc
    import concourse.mybir as mybir
    import concourse.tile as tile

    mm_dt = getattr(mybir.dt, mm_dtype_name)
    f32 = mybir.dt.float32

    nc = bacc.Bacc("TRN2", target_bir_lowering=False, debug=False, num_devices=N_CORES)
    data_in = nc.dram_tensor("data", [BPC, CIN, H, W], mm_dt, kind="ExternalInput").ap()
    # wt[ci, g*(9*128) + t*128 + co'] = weights[g*128+co', ci, ky, kx], t=ky*3+kx
    w_in = nc.dram_tensor("wt", [CIN, KH * KW * COUT], mm_dt, kind="ExternalInput").ap()
    out = nc.dram_tensor("out", [BPC, COUT, OH, OW], f32, kind="ExternalOutput").ap()
    WG = KH * KW * 128  # columns per co-half weight chunk

    with tile.TileContext(nc) as tc:
        with (
            tc.tile_pool(name="wpool", bufs=1) as wpool,
            tc.tile_pool(name="scr", bufs=1) as spool,
            tc.tile_pool(name="dpool", bufs=2) as dpool,
            tc.tile_pool(name="opool", bufs=6) as opool,
            tc.tile_pool(name="psum", bufs=8, space="PSUM") as ppool,
        ):
            # PE warm-up: the HAM clock gate holds the PE at 1.2 GHz until
            # one ~3.4us activity window has been busy, and the first ~9.5us
            # here are DMA-bound (preamble + weight/image loads). Run dummy
            # matmuls on memset scratch spanning that window so real
            # matmuls start at 2.4 GHz. (fp32 dummies don't work: they
            # lower to LOW_HIGH pairs, take 2-3x longer than budgeted, and
            # the sustained draw downclocks the whole stream.)
            # memset on the vector engine: its sequencer clears the
            # framework preamble earliest, so the warm-up (and with it
            # the HAM busy clock) starts as soon as possible
            if mm_dtype_name == "float32r":
                wscr = spool.tile([128, 512], f32)
                nc.vector.memset(wscr[:], 0.0)
                wsr = wscr[:].bitcast(mm_dt)
            else:
                wscr = spool.tile([128, 512], mm_dt)
                nc.vector.memset(wscr[:], 0.0)
                wsr = wscr[:]
            # the warm-up PSUM tile shares the main pool's slots (it is
            # long released by the time the 8th real group needs its bank)
            wps = ppool.tile([128, 512], f32, tag="ps")
            for _ in range(WARMUP_MM):
                nc.tensor.matmul(wps[:], wsr[:, :128], wsr[:], start=True, stop=True)

            # weight chunks: taps 0-2 of co-half 0 first, then the rest of
            # half 0, then half 1 (not needed until ~halfway through
            # image 0). Image-0 chunk 0 goes FIRST on the sync ring and
            # ALL weights go on the scalar ring: the two rings generate
            # descriptors concurrently, so the first row-group's inputs
            # land ~1.4us earlier than when serialized on one ring.
            wt_g0a = wpool.tile([CIN, 3 * 128], mm_dt, tag="wt0a")
            wt_g0b = wpool.tile([CIN, 6 * 128], mm_dt, tag="wt0b")
            wt_g1 = wpool.tile([CIN, WG], mm_dt, tag="wt1")

            def wslice(g, t):
                if g == 1:
                    return wt_g1[:, t * 128 : (t + 1) * 128]
                if t < 3:
                    return wt_g0a[:, t * 128 : (t + 1) * 128]
                return wt_g0b[:, (t - 3) * 128 : (t - 2) * 128]

            ct00 = dpool.tile([CIN, CHUNKS[0][1] * W], mm_dt, tag="d0")
            nc.sync.dma_start(
                ct00[:],
                data_in[0].rearrange("c h w -> c (h w)")[:, : CHUNKS[0][1] * W],
            )
            nc.scalar.dma_start(wt_g0a[:], w_in[:, : 3 * 128])
            nc.scalar.dma_start(wt_g0b[:], w_in[:, 3 * 128 : WG])

            dtiles = []
            for n in range(BPC):
                chunks = []
                flat = data_in[n].rearrange("c h w -> c (h w)")
                for ci, (c0, crows) in enumerate(CHUNKS):
                    if n == 0 and ci == 0:
                        chunks.append(ct00)
                        continue
                    ct = dpool.tile([CIN, crows * W], mm_dt, tag=f"d{ci}")
                    nc.sync.dma_start(ct[:], flat[:, c0 * W : (c0 + crows) * W])
                    chunks.append(ct)
                    if n == 0 and ci == len(CHUNKS) - 1:
                        nc.scalar.dma_start(wt_g1[:], w_in[:, WG:])
                dtiles.append(chunks)

            def rhs_for(chunks, r0, rows, t):
                ci = next(
                    i
                    for i, (c0, crows) in enumerate(CHUNKS)
                    if r0 >= c0 and r0 + rows + KH - 1 <= c0 + crows
                )
                hr0 = r0 - CHUNKS[ci][0]
                ky, kx = divmod(t, KW)
                c3 = chunks[ci][:].rearrange("p (r w) -> p r w", w=W)
                return c3[:, hr0 + ky : hr0 + ky + rows, kx : kx + OW]

            def out_rows(n, g, a, b):
                return out[n].rearrange("c h w -> c (h w)")[
                    g * 128 : (g + 1) * 128, a * OW : b * OW
                ]

            def evacuate(n, g, r, r0, rows, ps, tail):
                ot = opool.tile([128, 8 * OW], f32, tag="ot")
                if not tail:
                    # vector picks up the PSUM-done semaphore noticeably
                    # faster than scalar; alternate so neither engine backs
                    # up, and the stores stream on the scalar ring
                    if r % 2 == 0:
                        nc.vector.tensor_copy(ot[:, : rows * OW], ps[:])
                    else:
                        nc.scalar.copy(ot[:, : rows * OW], ps[:])
                    nc.scalar.dma_start(out_rows(n, g, r0, r0 + rows), ot[:, : rows * OW])
                else:
                    # the last two row-groups: copies on vector (fast sem
                    # pickup), stores on separate rings so the final 2-row
                    # store's descriptor gen + drain is all that remains
                    # after the last matmul group
                    nc.vector.tensor_copy(ot[:, : rows * OW], ps[:])
                    ring = nc.sync if rows == 2 else nc.scalar
                    ring.dma_start(out_rows(n, g, r0, r0 + rows), ot[:, : rows * OW])

            for n in range(BPC):
                chunks = dtiles[n]
                for g in range(COUT // 128):
                    is_last_ng = n == BPC - 1 and g == COUT // 128 - 1
                    groups = ROW_GROUPS_LAST if is_last_ng else ROW_GROUPS
                    for r, (r0, rows) in enumerate(groups):
                        ps = ppool.tile([128, rows * OW], f32, tag="ps")
                        for t in range(KH * KW):
                            nc.tensor.matmul(
                                ps[:],
                                wslice(g, t),
                                rhs_for(chunks, r0, rows, t),
                                start=(t == 0),
                                stop=(t == KH * KW - 1),
                            )
                        evacuate(n, g, r, r0, rows, ps, is_last_ng and rows < 8)
    nc.compile()
    return nc


def _get_nc(mm_dtype_name="bfloat16"):
    if mm_dtype_name not in _cache:
        _cache[mm_dtype_name] = build_nc(mm_dtype_name)
    return _cache[mm_dtype_name]


def _get_runner(mm_dtype_name="bfloat16"):
    """Build the 8-core PJRT executable once and cache it: repeat kernel()
    calls then skip bass2jax's per-call jit re-trace (~6s each)."""
    key = ("runner", mm_dtype_name)
    if key in _cache:
        return _cache[key]

    import jax
    import jax.core
    from jax.experimental.shard_map import shard_map
    from jax.sharding import Mesh, PartitionSpec

    import concourse.mybir as mybir
    from concourse import bass2jax

    nc = _get_nc(mm_dtype_name)
    bass2jax.install_neuronx_cc_hook()

    partition_name = nc.partition_id_tensor.name if nc.partition_id_tensor else None
    in_names, out_names, out_avals = [], [], []
    for alloc in nc.m.functions[0].allocations:
        if not isinstance(alloc, mybir.MemoryLocationSet):
            continue
        name = alloc.memorylocations[0].name
        if alloc.kind == "ExternalInput":
            if name != partition_name:
                in_names.append(name)
        elif alloc.kind == "ExternalOutput":
            out_names.append(name)
            out_avals.append(
                jax.core.ShapedArray(
                    tuple(alloc.tensor_shape), mybir.dt.np(alloc.dtype)
                )
            )
    n_params, n_outs = len(in_names), len(out_names)
    all_names = in_names + out_names
    if partition_name is not None:
        all_names = all_names + [partition_name]
    donate = tuple(range(n_params, n_params + n_outs))

    def _body(*args):
        operands = list(args)
        if partition_name is not None:
            operands.append(bass2jax.partition_id_tensor())
        outs = bass2jax._bass_exec_p.bind(
            *operands,
            out_avals=tuple(out_avals),
            in_names=tuple(all_names),
            out_names=tuple(out_names),
            lowering_input_output_aliases=(),
            sim_require_finite=True,
            sim_require_nnan=True,
            nc=nc,
        )
        return tuple(outs)

    devices = jax.devices()[:N_CORES]
    mesh = Mesh(np.asarray(devices), ("core",))
    sharded = jax.jit(
        shard_map(
            _body,
            mesh=mesh,
            in_specs=(PartitionSpec("core"),) * (n_params + n_outs),
            out_specs=(PartitionSpec("core"),) * n_outs,
            check_rep=False,
        ),
        donate_argnums=donate,
        keep_unused=True,
    )
    runner = (in_names, out_names, out_avals, sharded)
    _cache[key] = runner
    return runner


def _np_in_dtype(mm_dtype_name):
    if mm_dtype_name == "bfloat16":
        import ml_dtypes

        return ml_dtypes.bfloat16
    if mm_dtype_name == "float16":
        return np.float16
    return np.float32


def _prep_weights(weights, np_dt):
    # [co, ci, ky, kx] -> [ci][t=ky*3+kx][g][co'] -> [ci][g][t][co'] flat
    w4 = np.asarray(weights, dtype=np.float32).transpose(1, 2, 3, 0)  # ci,ky,kx,co
    w4 = w4.reshape(CIN, KH * KW, COUT // 128, 128).transpose(0, 2, 1, 3)
    return np.ascontiguousarray(w4, dtype=np_dt).reshape(CIN, KH * KW * COUT)


def kernel(data: np.ndarray, weights: np.ndarray, _dtype="bfloat16") -> np.ndarray:
    np_dt = _np_in_dtype(_dtype)
    data = np.ascontiguousarray(np.asarray(data), dtype=np_dt)
    wt = _prep_weights(weights, np_dt)

    in_names, out_names, out_avals, sharded = _get_runner(_dtype)
    # shard_map splits axis 0 across the 8 cores: the global batch-sharded
    # arrays are exactly the full input (batch 32 -> 4 per core) and the
    # per-core-replicated weights tiled 8x on axis 0.
    globals_ = {
        "data": data.reshape(N_CORES * BPC, CIN, H, W),
        "wt": np.tile(wt, (N_CORES, 1)),
    }
    args = [globals_[n] for n in in_names] + [
        np.zeros((N_CORES * av.shape[0], *av.shape[1:]), av.dtype)
        for av in out_avals
    ]
    outs = sharded(*args)
    return np.asarray(outs[out_names.index("out")])


# revision 18
# speedup vs baseline: 1.0185x; 1.0029x over previous
# Trainium2 Bass kernel for nn_CustomConv2D_57200374448719:
#   data [32,128,64,64] f32 (NCHW) conv weights [256,128,3,3] (OIHW),
#   VALID, stride 1 -> out [32,256,62,62] f32.
#
# Strategy: data-parallel over batch across 8 NeuronCores (4 images per
# core), weights replicated. Per core, implicit GEMM with C_in=128 on the
# SBUF partition axis: for each image / C_out half (128) / group of 8
# output rows, accumulate 9 matmuls (one per 3x3 tap, K=128, N=rows*62)
# into one PSUM bank. The shifted conv windows are strided access
# patterns on the resident image tiles (3D moving AP [128, rows, 62],
# row stride 64), so no im2col copy is materialized and no garbage
# columns are streamed. Matmuls run in bfloat16 by default: bf16 lowers
# to separate LDWEIGHTS+MATMUL with fast-weight-load, and the PE's
# reorder window pulls the next LDWEIGHTS into the background weight
# buffer during the current matmul's streaming - the stationary loads
# that cost ~20% extra on the fp32r self-loading path are fully hidden.
# Accumulation is fp32 in PSUM; measured rel err ~2e-3.
#
# Startup-latency hiding: weights are loaded as two per-co-half chunks
# and each image as halo'd row chunks, all on the sync-engine HWDGE
# ring, ordered so the first row-group's dependencies (first weight
# chunk + first image chunk) land as early as possible. While those
# DMAs are in flight, 8 dummy matmuls on a memset scratch tile keep the
# PE busy so the HAM clock gate releases (1.2 -> 2.4 GHz) right as real
# work begins. Output stores go on the scalar-engine HWDGE ring per
# row-group so they stream out during compute; the final row-group's
# store is split in half across the scalar and sync rings to shorten
# the drain tail.
import numpy as np

N_CORES = 8
B, CIN, H, W = 32, 128, 64, 64
COUT, KH, KW = 256, 3, 3
OH, OW = H - KH + 1, W - KW + 1  # 62, 62
BPC = B // N_CORES  # images per core
ROW_GROUPS = [(r0, min(8, OH - r0)) for r0 in range(0, OH, 8)]  # 7x8 + 1x6
# image row chunks (with conv halo): rows [0,10) serves row-group 0,
# [8,18) serves 1, [16,34) serves 2-3, [32,64) serves 4-7. The first two
# are small so the first matmuls' DMA dependencies land early. Startup
# is input-bandwidth-bound (~300 GB/s aggregate + ~1.7us completion-
# semaphore latency per transfer), so finer staging is zero-sum: it
# releases the first matmul earlier but starves later groups instead.
CHUNKS = [(0, 10), (8, 10), (16, 18), (32, 32)]
# The first real matmul can't release before ~10.7us: first DMA trigger
# (~6.5, after the framework preamble) + ~1.5us HWDGE arming + ~1us
# descriptor drain + ~1.7us completion-semaphore latency. Dummy matmuls
# bridge PE-busy start (~7.7us, after the scratch memset) to that point
# so the HAM clock gate is released (1.2 -> 2.4 GHz) when real work
# starts. 8 emitted = 7 executed x ~0.43us cold.
WARMUP_MM = 8
# the very last (image, co-half) splits its final 6-row group into 4+2
# so the kernel-end store drain is 2 rows (63KB) instead of 6 (190KB)
ROW_GROUPS_LAST = ROW_GROUPS[:-1] + [(56, 4), (60, 2)]

_cache = {}


def build_nc(mm_dtype_name="bfloat16"):
    import concourse.bacc as bacc
    import concourse.mybir as mybir
    import concourse.tile as tile

    mm_dt = getattr(mybir.dt, mm_dtype_name)
    f32 = mybir.dt.float32

    nc = bacc.Bacc("TRN2", target_bir_lowering=False, debug=False, num_devices=N_CORES)
    data_in = nc.dram_tensor("data", [BPC, CIN, H, W], mm_dt, kind="ExternalInput").ap()
    # wt[ci, g*(9*128) + t*128 + co'] = weights[g*128+co', ci, ky, kx], t=ky*3+kx
    w_in = nc.dram_tensor("wt", [CIN, KH * KW * COUT], mm_dt, kind="ExternalInput").ap()
    out = nc.dram_tensor("out", [BPC, COUT, OH, OW], f32, kind="ExternalOutput").ap()
    WG = KH * KW * 128  # columns per co-half weight chunk

    with tile.TileContext(nc) as tc:
        with (
            tc.tile_pool(name="wpool", bufs=1) as wpool,
            tc.tile_pool(name="scr", bufs=1) as spool,
            tc.tile_pool(name="dpool", bufs=2) as dpool,
            tc.tile_pool(name="opool", bufs=6) as opool,
            tc.tile_pool(name="psum", bufs=8, space="PSUM") as ppool,
        ):
            # PE warm-up: the HAM clock gate holds the PE at 1.2 GHz until
            # one ~3.4us activity window has been busy, and the first ~9.5us
            # here are DMA-bound (preamble + weight/image loads). Run dummy
            # matmuls on memset scratch spanning that window so real
            # matmuls start at 2.4 GHz. (fp32 dummies don't work: they
            # lower to LOW_HIGH pairs, take 2-3x longer than budgeted, and
            # the sustained draw downclocks the whole stream.)
            # memset on the vector engine: its sequencer clears the
            # framework preamble earliest, so the warm-up (and with it
            # the HAM busy clock) starts as soon as possible
            if mm_dtype_name == "float32r":
                wscr = spool.tile([128, 512], f32)
                nc.vector.memset(wscr[:], 0.0)
                wsr = wscr[:].bitcast(mm_dt)
            else:
                wscr = spool.tile([128, 512], mm_dt)
                nc.vector.memset(wscr[:], 0.0)
                wsr = wscr[:]
            # the warm-up PSUM tile shares the main pool's slots (it is
            # long released by the time the 8th real group needs its bank)
            wps = ppool.tile([128, 512], f32, tag="ps")
            for _ in range(WARMUP_MM):
                nc.tensor.matmul(wps[:], wsr[:, :128], wsr[:], start=True, stop=True)

            # weight chunks: taps 0-2 of co-half 0 first, then the rest of
            # half 0, then half 1 (not needed until ~halfway through
            # image 0). Image-0 chunk 0 goes FIRST on the sync ring and
            # ALL weights go on the scalar ring: the two rings generate
            # descriptors concurrently, so the first row-group's inputs
            # land ~1.4us earlier than when serialized on one ring.
            wt_g0a = wpool.tile([CIN, 3 * 128], mm_dt, tag="wt0a")
            wt_g0b = wpool.tile([CIN, 3 * 128], mm_dt, tag="wt0b")
            wt_g0c = wpool.tile([CIN, 3 * 128], mm_dt, tag="wt0c")
            wt_g1 = wpool.tile([CIN, WG], mm_dt, tag="wt1")

            def wslice(g, t):
                if g == 1:
                    return wt_g1[:, t * 128 : (t + 1) * 128]
                wt = (wt_g0a, wt_g0b, wt_g0c)[t // 3]
                return wt[:, (t % 3) * 128 : (t % 3 + 1) * 128]

            ct00 = dpool.tile([CIN, CHUNKS[0][1] * W], mm_dt, tag="d0")
            nc.sync.dma_start(
                ct00[:],
                data_in[0].rearrange("c h w -> c (h w)")[:, : CHUNKS[0][1] * W],
            )
            # co-half 0's weights as three 96 KB tap-triples: taps 6-8's
            # bytes leave the queue window ahead of image chunk 1, whose
            # completion gates row-group 1 on bandwidth-pinched runs
            nc.scalar.dma_start(wt_g0a[:], w_in[:, : 3 * 128])
            nc.scalar.dma_start(wt_g0b[:], w_in[:, 3 * 128 : 6 * 128])
            nc.scalar.dma_start(wt_g0c[:], w_in[:, 6 * 128 : WG])

            dtiles = []
            for n in range(BPC):
                chunks = []
                flat = data_in[n].rearrange("c h w -> c (h w)")
                for ci, (c0, crows) in enumerate(CHUNKS):
                    if n == 0 and ci == 0:
                        chunks.append(ct00)
                        continue
                    ct = dpool.tile([CIN, crows * W], mm_dt, tag=f"d{ci}")
                    nc.sync.dma_start(ct[:], flat[:, c0 * W : (c0 + crows) * W])
                    chunks.append(ct)
                    if n == 0 and ci == len(CHUNKS) - 1:
                        nc.scalar.dma_start(wt_g1[:], w_in[:, WG:])
                dtiles.append(chunks)

            def rhs_for(chunks, r0, rows, t):
                ci = next(
                    i
                    for i, (c0, crows) in enumerate(CHUNKS)
                    if r0 >= c0 and r0 + rows + KH - 1 <= c0 + crows
                )
                hr0 = r0 - CHUNKS[ci][0]
                ky, kx = divmod(t, KW)
                c3 = chunks[ci][:].rearrange("p (r w) -> p r w", w=W)
                return c3[:, hr0 + ky : hr0 + ky + rows, kx : kx + OW]

            def out_rows(n, g, a, b):
                return out[n].rearrange("c h w -> c (h w)")[
                    g * 128 : (g + 1) * 128, a * OW : b * OW
                ]

            def evacuate(n, g, r, r0, rows, ps, tail):
                ot = opool.tile([128, 8 * OW], f32, tag="ot")
                if not tail:
                    # vector picks up the PSUM-done semaphore noticeably
                    # faster than scalar; alternate so neither engine backs
                    # up, and the stores stream on the scalar ring
                    if r % 2 == 0:
                        nc.vector.tensor_copy(ot[:, : rows * OW], ps[:])
                    else:
                        nc.scalar.copy(ot[:, : rows * OW], ps[:])
                    nc.scalar.dma_start(out_rows(n, g, r0, r0 + rows), ot[:, : rows * OW])
                else:
                    # the last two row-groups: copies on vector (fast sem
                    # pickup), stores on separate rings so the final 2-row
                    # store's descriptor gen + drain is all that remains
                    # after the last matmul group
                    nc.vector.tensor_copy(ot[:, : rows * OW], ps[:])
                    ring = nc.sync if rows == 2 else nc.scalar
                    ring.dma_start(out_rows(n, g, r0, r0 + rows), ot[:, : rows * OW])

            for n in range(BPC):
                chunks = dtiles[n]
                for g in range(COUT // 128):
                    is_last_ng = n == BPC - 1 and g == COUT // 128 - 1
                    groups = ROW_GROUPS_LAST if is_last_ng else ROW_GROUPS
                    for r, (r0, rows) in enumerate(groups):
                        ps = ppool.tile([128, rows * OW], f32, tag="ps")
                        for t in range(KH * KW):
                            nc.tensor.matmul(
                                ps[:],
                                wslice(g, t),
                                rhs_for(chunks, r0, rows, t),
                                start=(t == 0),
                                stop=(t == KH * KW - 1),
                            )
                        evacuate(n, g, r, r0, rows, ps, is_last_ng and rows < 8)
    nc.compile()
    return nc


def _get_nc(mm_dtype_name="bfloat16"):
    if mm_dtype_name not in _cache:
        _cache[mm_dtype_name] = build_nc(mm_dtype_name)
    return _cache[mm_dtype_name]


def _get_runner(mm_dtype_name="bfloat16"):
    """Build the 8-core PJRT executable once and cache it: repeat kernel()
    calls then skip bass2jax's per-call jit re-trace (~6s each)."""
    key = ("runner", mm_dtype_name)
    if key in _cache:
        return _cache[key]

    import jax
    import jax.core
    from jax.experimental.shard_map import shard_map
    from jax.sharding import Mesh, PartitionSpec

    import concourse.mybir as mybir
    from concourse import bass2jax

    nc = _get_nc(mm_dtype_name)
    bass2jax.install_neuronx_cc_hook()

    partition_name = nc.partition_id_tensor.name if nc.partition_id_tensor else None
    in_names, out_names, out_avals = [], [], []
    for alloc in nc.m.functions[0].allocations:
        if not isinstance(alloc, mybir.MemoryLocationSet):
            continue
        name = alloc.memorylocations[0].name
        if alloc.kind == "ExternalInput":
            if name != partition_name:
                in_names.append(name)
        elif alloc.kind == "ExternalOutput":
            out_names.append(name)
            out_avals.append(
                jax.core.ShapedArray(
                    tuple(alloc.tensor_shape), mybir.dt.np(alloc.dtype)
                )
            )
    n_params, n_outs = len(in_names), len(out_names)
    all_names = in_names + out_names
    if partition_name is not None:
        all_names = all_names + [partition_name]
    donate = tuple(range(n_params, n_params + n_outs))

    def _body(*args):
        operands = list(args)
        if partition_name is not None:
            operands.append(bass2jax.partition_id_tensor())
        outs = bass2jax._bass_exec_p.bind(
            *operands,
            out_avals=tuple(out_avals),
            in_names=tuple(all_names),
            out_names=tuple(out_names),
            lowering_input_output_aliases=(),
            sim_require_finite=True,
            sim_require_nnan=True,
            nc=nc,
        )
        return tuple(outs)

    devices = jax.devices()[:N_CORES]
    mesh = Mesh(np.asarray(devices), ("core",))
    sharded = jax.jit(
        shard_map(
            _body,
            mesh=mesh,
            in_specs=(PartitionSpec("core"),) * (n_params + n_outs),
            out_specs=(PartitionSpec("core"),) * n_outs,
            check_rep=False,
        ),
        donate_argnums=donate,
        keep_unused=True,
    )
    runner = (in_names, out_names, out_avals, sharded)
    _cache[key] = runner
    return runner


def _np_in_dtype(mm_dtype_name):
    if mm_dtype_name == "bfloat16":
        import ml_dtypes

        return ml_dtypes.bfloat16
    if mm_dtype_name == "float16":
        return np.float16
    return np.float32


def _prep_weights(weights, np_dt):
    # [co, ci, ky, kx] -> [ci][t=ky*3+kx][g][co'] -> [ci][g][t][co'] flat
    w4 = np.asarray(weights, dtype=np.float32).transpose(1, 2, 3, 0)  # ci,ky,kx,co
    w4 = w4.reshape(CIN, KH * KW, COUT // 128, 128).transpose(0, 2, 1, 3)
    return np.ascontiguousarray(w4, dtype=np_dt).reshape(CIN, KH * KW * COUT)


def kernel(data: np.ndarray, weights: np.ndarray, _dtype="bfloat16") -> np.ndarray:
    np_dt = _np_in_dtype(_dtype)
    data = np.ascontiguousarray(np.asarray(data), dtype=np_dt)
    wt = _prep_weights(weights, np_dt)

    in_names, out_names, out_avals, sharded = _get_runner(_dtype)
    # shard_map splits axis 0 across the 8 cores: the global batch-sharded
    # arrays are exactly the full input (batch 32 -> 4 per core) and the
    # per-core-replicated weights tiled 8x on axis 0.
    globals_ = {
        "data": data.reshape(N_CORES * BPC, CIN, H, W),
        "wt": np.tile(wt, (N_CORES, 1)),
    }
    args = [globals_[n] for n in in_names] + [
        np.zeros((N_CORES * av.shape[0], *av.shape[1:]), av.dtype)
        for av in out_avals
    ]
    outs = sharded(*args)
    return np.asarray(outs[out_names.index("out")])
